# revision 39
# baseline (speedup 1.0000x reference)
"""Trainium2 Bass kernel for nn_Critic (B=128, N=2048, H=128), data-parallel over 8 cores.

Key algebraic reduction: the encoders are rank-1/2 in H, so every [B,H,N]
conv inside the attention blocks collapses to a rank-4 structure:

    tanh-arg  z[b,h,n] = sum_j A_i[j,h] * X[b,j,n] + Q_i[b,h]
      X rows j: [static[:,0], static[:,1], state_init, static_ch_l]
      A_i rows: [Wr_i@W_st[:,0], Wr_i@W_st[:,1], Wd_i@W_dyn[:,0], Wc_i@W_ch[:,0]]
      Q_i[b,h] = (hy @ Wq_i.T)[b,h] + cvec_i[h]
    u[b,n]   = sum_h v_i[h] tanh(z[b,h,n])
    probs    = softmax(u, axis=n)   (|u| <= sum|v| ~ 9, no max-subtraction)
    y_j[b]   = sum_n probs[b,n] * Xec[b,j,n]
    q_{i+1}[h,b] = G_i[j,h] . ysb[j,b]   with G_i = Aec_i @ Wq_{i+1}^T
    out = relu(hy@W1.T + b1) @ W2.T + b2,  hy = Aec_2^T ysb

Schedule (per core, BL=16 batches):
  - all big matmuls in bf16; u reduced via sliding-window one-hot v columns
    into one PSUM bank laid out [64, 512] with partition p = 4*b + q2
  - software pipeline: u matmuls lag two half-batches behind z (zpool bufs=3)
    so the in-order PE queue never waits on tanh
  - block i+1's weight prep (transposes, A rows, G) is emitted in block i's
    softmax tail, filling the PE gap there
  - X loads/casts use the p=4b+q2 layout so each stage is a single DMA
"""

import os
import sys

try:
    import concourse.bass  # noqa: F401  (present on the axon PYTHONPATH already)
except ImportError:
    for _p in ("/root/.axon_site/_ro/trn_rl_repo", "/opt/trn_rl_repo"):
        if _p not in sys.path:
            sys.path.append(_p)

import numpy as np

import concourse.bass as bass
import concourse.bacc as bacc
import concourse.mybir as mybir
import concourse.tile as tile
from concourse.bass_utils import run_bass_kernel_spmd
from concourse.masks import make_identity

B, N, H = 128, 2048, 128
NCORES = 8
BL = B // NCORES  # 16 batches per core
NQ = 4  # 512-wide n-chunks; softmax layout partition p = 4*b + q2
NC = N // NQ  # 512

F32 = mybir.dt.float32
BF16 = mybir.dt.bfloat16
AF = mybir.ActivationFunctionType
ALU = mybir.AluOpType

_PROGRAM = None


def _build_program():
    nc = bacc.Bacc("TRN2", target_bir_lowering=False, debug=False)

    d_state = nc.declare_dram_parameter("state_init", [BL, 1, N], F32, isOutput=False)
    d_static = nc.declare_dram_parameter("static", [BL, 2, N], F32, isOutput=False)
    d_ch = nc.declare_dram_parameter("static_ch_l", [BL, 1, N], F32, isOutput=False)
    d_Wdyn = nc.declare_dram_parameter("W_dyn", [H, 1], F32, isOutput=False)
    d_bdyn = nc.declare_dram_parameter("b_dyn", [H], F32, isOutput=False)
    d_Wst = nc.declare_dram_parameter("W_st", [H, 2], F32, isOutput=False)
    d_bst = nc.declare_dram_parameter("b_st", [H], F32, isOutput=False)
    d_Wch = nc.declare_dram_parameter("W_ch", [H, 1], F32, isOutput=False)
    d_bch = nc.declare_dram_parameter("b_ch", [H], F32, isOutput=False)
    d_v = nc.declare_dram_parameter("v", [3, H], F32, isOutput=False)
    d_Wd = nc.declare_dram_parameter("Wd", [3, H, H], F32, isOutput=False)
    d_bd = nc.declare_dram_parameter("bd", [3, H], F32, isOutput=False)
    d_Wc = nc.declare_dram_parameter("Wc", [3, H, H], F32, isOutput=False)
    d_bc = nc.declare_dram_parameter("bc", [3, H], F32, isOutput=False)
    d_Wr = nc.declare_dram_parameter("Wr", [3, H, H], F32, isOutput=False)
    d_br = nc.declare_dram_parameter("br", [3, H], F32, isOutput=False)
    d_Wq = nc.declare_dram_parameter("Wq", [3, H, H], F32, isOutput=False)
    d_bq = nc.declare_dram_parameter("bq", [3, H], F32, isOutput=False)
    d_W1 = nc.declare_dram_parameter("W1", [H, H], F32, isOutput=False)
    d_b1 = nc.declare_dram_parameter("b1", [H], F32, isOutput=False)
    d_W2 = nc.declare_dram_parameter("W2", [1, H], F32, isOutput=False)
    d_b2 = nc.declare_dram_parameter("b2", [1], F32, isOutput=False)
    d_out = nc.declare_dram_parameter("out", [BL, 1], F32, isOutput=True)

    with tile.TileContext(nc) as tc:
        _emit(nc, tc, locals())
    nc.compile()
    return nc


STAGE = int(os.environ.get("KSTAGE", "0"))


def _emit(nc, tc, d):
    from contextlib import ExitStack

    ctx = ExitStack()
    with ctx:
        consts = ctx.enter_context(tc.tile_pool(name="consts", bufs=1))
        stage = ctx.enter_context(tc.tile_pool(name="stage", bufs=2))
        tpool = ctx.enter_context(tc.tile_pool(name="tanh", bufs=2))
        spool = ctx.enter_context(tc.tile_pool(name="spool", bufs=2))
        zpool = ctx.enter_context(tc.tile_pool(name="zp", bufs=3, space="PSUM"))
        upool = ctx.enter_context(tc.tile_pool(name="up", bufs=1, space="PSUM"))
        psmall = ctx.enter_context(tc.tile_pool(name="psm", bufs=1, space="PSUM"))

        # DMA issue queues: the X chain rides the idle ACT queue (its consumer,
        # the first tanh, queues behind it anyway); weights and row writes use
        # sync's fast HWDGE path instead of gpsimd's ~1.4us-per-DMA SWDGE.
        dma_x = nc.scalar.dma_start
        dma_w = nc.sync.dma_start
        dma_r = nc.sync.dma_start

        # ================= X loads / casts (critical path) =================
        # layout p = 4*b + q2 makes every stage a single contiguous-order DMA
        Xec4 = consts.tile([64, 3, NC], F32, tag="Xec4")
        ec_srcs = [
            d["d_static"].ap()[:, 0, :],
            d["d_static"].ap()[:, 1, :],
            d["d_ch"].ap()[:, 0, :],
        ]
        for j, src in enumerate(ec_srcs):
            dma_x(out=Xec4[:, j, :], in_=src.rearrange("b (q c) -> b q c", q=NQ))
        Xst = consts.tile([64, NC], F32, tag="Xst")
        dma_x(out=Xst, in_=d["d_state"].ap()[:, 0, :].rearrange("b (q c) -> b q c", q=NQ))

        Xb4 = consts.tile([64, 4, NC], BF16, tag="Xb4")
        nc.vector.tensor_copy(out=Xb4[:, 0, :], in_=Xec4[:, 0, :])
        nc.vector.tensor_copy(out=Xb4[:, 1, :], in_=Xec4[:, 1, :])
        nc.vector.tensor_copy(out=Xb4[:, 2, :], in_=Xst)
        nc.vector.tensor_copy(out=Xb4[:, 3, :], in_=Xec4[:, 2, :])
        Xhat = consts.tile([4, BL, N], BF16, tag="Xhat")
        for j in range(4):
            dma_x(out=Xhat[j : j + 1, :, :], in_=Xb4[:, j, :])

        # ================= small constants =================
        identity = consts.tile([128, 128], F32, tag="identity")
        make_identity(nc, identity)

        wdyn = consts.tile([H, 1], F32, tag="wdyn")
        dma_w(out=wdyn, in_=d["d_Wdyn"].ap())
        wst = consts.tile([H, 2], F32, tag="wst")
        dma_w(out=wst, in_=d["d_Wst"].ap())
        wch = consts.tile([H, 1], F32, tag="wch")
        dma_w(out=wch, in_=d["d_Wch"].ap())
        bdyn = consts.tile([H, 1], F32, tag="bdyn")
        dma_w(out=bdyn, in_=d["d_bdyn"].ap().rearrange("(p one) -> p one", one=1))
        bst = consts.tile([H, 1], F32, tag="bst")
        dma_w(out=bst, in_=d["d_bst"].ap().rearrange("(p one) -> p one", one=1))
        bch = consts.tile([H, 1], F32, tag="bch")
        dma_w(out=bch, in_=d["d_bch"].ap().rearrange("(p one) -> p one", one=1))
        b1col = consts.tile([H, 1], F32, tag="b1col")
        dma_w(out=b1col, in_=d["d_b1"].ap().rearrange("(p one) -> p one", one=1))
        b2sb = consts.tile([1, 1], F32, tag="b2sb")
        dma_w(out=b2sb, in_=d["d_b2"].ap().rearrange("(one o2) -> one o2", one=1))

        # v columns; sliding-window one-hot lhsT (v_i at window column K=4b+q2)
        vcol = consts.tile([H, 3], F32, tag="vcol")
        for i in range(3):
            dma_w(out=vcol[:, i : i + 1], in_=d["d_v"][i : i + 1, :].rearrange("one h -> h one"))
        Vwin = consts.tile([128, 3, 127], BF16, tag="Vwin")
        nc.vector.memset(Vwin, 0.0)
        for i in range(3):
            nc.vector.tensor_copy(out=Vwin[:, i, 63:64], in_=vcol[:, i : i + 1])

        # selector: sel64[4b+q2, b'] = (b == b') <=> 0 <= p - 4*b' <= 3
        sel64 = consts.tile([64, BL], F32, tag="sel64")
        nc.gpsimd.memset(sel64, 1.0)
        nc.gpsimd.affine_select(
            out=sel64, in_=sel64, compare_op=ALU.is_ge,
            fill=0.0, base=0, pattern=[[-NQ, BL]], channel_multiplier=1,
        )
        nc.gpsimd.affine_select(
            out=sel64, in_=sel64, compare_op=ALU.is_ge,
            fill=0.0, base=NQ - 1, pattern=[[NQ, BL]], channel_multiplier=-1,
        )

        WdT = consts.tile([H, 3, H], F32, tag="WdT")
        WcT = consts.tile([H, 3, H], F32, tag="WcT")
        WrT = consts.tile([H, 3, H], F32, tag="WrT")
        WqT = consts.tile([H, 2, H], F32, tag="WqT")  # only blocks 1,2 use Wq
        W1T = consts.tile([H, H], F32, tag="W1T")
        W2col = consts.tile([H, 1], F32, tag="W2col")
        cvec = consts.tile([H, 3], F32, tag="cvec")
        Aall = consts.tile([4, 3, H], BF16, tag="Aall")
        Aec = consts.tile([4, 3, H], F32, tag="Aec")
        Gsb = consts.tile([4, 2, H], F32, tag="Gsb")

        def load_T(dsrc_ap, dst_ap):
            st = stage.tile([H, H], F32, tag="wstage")
            dma_w(out=st, in_=dsrc_ap)
            ps = psmall.tile([H, H], F32, tag="ps")
            nc.tensor.transpose(ps, st, identity)
            nc.vector.tensor_copy(out=dst_ap, in_=ps)

        def prep_block_weights(i):
            """Transposes, cvec/cec, A rows for block i (emitted lazily so the
            PE work lands in the previous block's softmax-tail gap)."""
            load_T(d["d_Wd"][i], WdT[:, i, :])
            load_T(d["d_Wc"][i], WcT[:, i, :])
            load_T(d["d_Wr"][i], WrT[:, i, :])

            bstack = stage.tile([4, H], F32, tag="bstack")
            for j, dsrc in enumerate((d["d_bd"], d["d_bc"], d["d_br"], d["d_bq"])):
                dma_w(out=bstack[j : j + 1, :], in_=dsrc[i : i + 1, :])
            bsT = psmall.tile([H, 4], F32, tag="ps")
            nc.tensor.transpose(bsT, bstack, identity[0:4, 0:4])
            bsT_sb = stage.tile([H, 4], F32, tag="bsT")
            nc.vector.tensor_copy(out=bsT_sb, in_=bsT)

            cps = psmall.tile([H, 1], F32, tag="ps")
            nc.tensor.matmul(cps, WrT[:, i, :], bst, start=True, stop=False)
            nc.tensor.matmul(cps, WdT[:, i, :], bdyn, start=False, stop=False)
            nc.tensor.matmul(cps, WcT[:, i, :], bch, start=False, stop=True)
            bsum = stage.tile([H, 1], F32, tag="bsum")
            nc.vector.tensor_reduce(out=bsum, in_=bsT_sb, axis=mybir.AxisListType.X, op=ALU.add)
            nc.vector.tensor_tensor(out=cvec[:, i : i + 1], in0=cps, in1=bsum, op=ALU.add)

            ceps = psmall.tile([H, 1], F32, tag="ps")
            nc.tensor.matmul(ceps, WrT[:, i, :], bst, start=True, stop=False)
            nc.tensor.matmul(ceps, WcT[:, i, :], bch, start=False, stop=True)
            cec_col = stage.tile([H, 1], F32, tag="ceccol")
            nc.vector.tensor_tensor(out=cec_col, in0=bsT_sb[:, 2:3], in1=bsT_sb[:, 1:2], op=ALU.add)
            nc.vector.tensor_tensor(out=cec_col, in0=cec_col, in1=ceps, op=ALU.add)
            cecT = psmall.tile([1, H], F32, tag="ps")
            nc.tensor.transpose(cecT, cec_col, identity)
            cecT_sb = stage.tile([1, H], F32, tag="rowsb")
            nc.vector.tensor_copy(out=cecT_sb, in_=cecT)
            dma_r(out=Aec[3:4, i, :], in_=cecT_sb)

            row_specs = [
                (wst[:, 0:1], WrT[:, i, :], 0, 0),
                (wst[:, 1:2], WrT[:, i, :], 1, 1),
                (wdyn[:, 0:1], WdT[:, i, :], 2, None),
                (wch[:, 0:1], WcT[:, i, :], 3, 2),
            ]
            for col, wt, jrow, ecrow in row_specs:
                rps = psmall.tile([1, H], F32, tag="ps")
                nc.tensor.matmul(rps, col, wt, start=True, stop=True)
                rsbb = stage.tile([1, H], BF16, tag="rowsbb")
                nc.vector.tensor_copy(out=rsbb, in_=rps)
                dma_r(out=Aall[jrow : jrow + 1, i, :], in_=rsbb)
                if ecrow is not None:
                    rsb = stage.tile([1, H], F32, tag="rowsb")
                    nc.vector.tensor_copy(out=rsb, in_=rps)
                    dma_r(out=Aec[ecrow : ecrow + 1, i, :], in_=rsb)

        def prep_G(i):
            """G_i = Aec_i @ Wq_{i+1}^T (needs prep_block_weights(i) and WqT)."""
            load_T(d["d_Wq"][i + 1], WqT[:, i, :])
            aet = psmall.tile([H, 4], F32, tag="ps")
            nc.tensor.transpose(aet, Aec[:, i, :], identity[0:4, 0:4])
            aet_sb = stage.tile([H, 4], F32, tag="aetsb")
            nc.vector.tensor_copy(out=aet_sb, in_=aet)
            gps = psmall.tile([4, H], F32, tag="ps")
            nc.tensor.matmul(gps, aet_sb, WqT[:, i, :], start=True, stop=True)
            nc.vector.tensor_copy(out=Gsb[:, i, :], in_=gps)

        def prep_head():
            load_T(d["d_W1"].ap(), W1T)
            w2row = stage.tile([1, H], F32, tag="w2row")
            dma_w(out=w2row, in_=d["d_W2"].ap())
            ps = psmall.tile([H, 1], F32, tag="ps")
            nc.tensor.transpose(ps, w2row, identity[0:1, 0:1])
            nc.vector.tensor_copy(out=W2col, in_=ps)

        prep_block_weights(0)

        def diag(src_row):
            nc.sync.dma_start(out=d["d_out"].ap().rearrange("b one -> one b"), in_=src_row)

        if STAGE == 1:
            dg = spool.tile([1, BL], F32, tag="dg")
            nc.vector.tensor_copy(out=dg, in_=Aall[0:1, 0, 0:BL].bitcast(BF16))
            diag(dg)
            return

        # ================= main: 3 attention blocks =================
        def emit_z(i, idx):
            b, h = idx // 2, idx % 2
            zp = zpool.tile([128, 1024], F32, tag="z")
            base = h * 1024
            nc.tensor.matmul(zp[:, 0:512], Aall[:, i, :], Xhat[:, b, base : base + 512],
                             start=True, stop=True)
            nc.tensor.matmul(zp[:, 512:1024], Aall[:, i, :], Xhat[:, b, base + 512 : base + 1024],
                             start=True, stop=True)
            return zp

        def emit_tanh(i, idx, zp, Qsb):
            b, h = idx // 2, idx % 2
            bias_ap = cvec[:, 0:1] if i == 0 else Qsb[:, b : b + 1]
            Th = tpool.tile([128, 1024], BF16, tag=f"T{h}")
            nc.scalar.activation(out=Th, in_=zp, func=AF.Tanh, bias=bias_ap, scale=1.0)
            return Th

        def emit_u(i, idx, Th, u4ps, first):
            b, h = idx // 2, idx % 2
            for c in range(2):
                K = NQ * b + 2 * h + c
                nc.tensor.matmul(
                    u4ps,
                    Vwin[:, i, 63 - K : 127 - K],
                    Th[:, NC * c : NC * (c + 1)],
                    start=(first and c == 0),
                    stop=True,
                    skip_group_check=True,
                )

        Qsb = None
        ysb = None
        prez = []  # z tiles of the next block, pre-issued to fill the tail gap
        for i in range(3):
            u4ps = upool.tile([64, NC], F32, tag="u4")
            pend = []
            for idx in range(2 * BL):
                zp = prez.pop(0) if prez else emit_z(i, idx)
                Th = emit_tanh(i, idx, zp, Qsb)
                pend.append((idx, Th))
                if len(pend) > 2:
                    pidx, pTh = pend.pop(0)
                    emit_u(i, pidx, pTh, u4ps, first=(pidx == 0))
            for pidx, pTh in pend:
                emit_u(i, pidx, pTh, u4ps, first=(pidx == 0))

            if STAGE == 2 and i == 0:
                dg = spool.tile([1, BL], F32, tag="dg")
                nc.vector.tensor_copy(out=dg, in_=u4ps[0:1, 0:BL])
                diag(dg)
                return

            # next block's weight prep + first z matmuls fill the PE gap
            # during the tail (their tanh is only emitted after exp, post-Qsb)
            if i == 0:
                prep_block_weights(1)
                prep_G(0)
            elif i == 1:
                prep_block_weights(2)
                prep_G(1)
                prep_head()
            if i < 2:
                prez = [emit_z(i + 1, k) for k in range(3)]

            # ---- softmax + y reduction tail (layout p = 4b + q2) ----
            E4 = spool.tile([64, NC], F32, tag="E4")
            S4 = spool.tile([64, 1], F32, tag="S4")
            nc.scalar.activation(out=E4, in_=u4ps, func=AF.Exp, accum_out=S4)
            Y4 = spool.tile([64, 4], F32, tag="Y4")
            for j in range(3):
                scr = spool.tile([64, NC], F32, tag="scr")
                nc.vector.tensor_tensor(out=scr, in0=E4, in1=Xec4[:, j, :], op=ALU.mult)
                nc.vector.tensor_reduce(out=Y4[:, j : j + 1], in_=scr, axis=mybir.AxisListType.X, op=ALU.add)
            nc.vector.tensor_copy(out=Y4[:, 3:4], in_=S4)
            yall = psmall.tile([BL, 4], F32, tag="ps")
            nc.tensor.matmul(yall, sel64, Y4, start=True, stop=True)
            recS = spool.tile([BL, 1], F32, tag="recS")
            nc.vector.reciprocal(out=recS, in_=yall[:, 3:4])
            ytile = spool.tile([BL, 4], F32, tag="ytile")
            nc.vector.memset(ytile[:, 3:4], 1.0)
            nc.vector.tensor_scalar(out=ytile[:, 0:3], in0=yall[:, 0:3], scalar1=recS,
                                    scalar2=None, op0=ALU.mult)
            yT = psmall.tile([4, BL], F32, tag="ps")
            nc.tensor.transpose(yT, ytile, identity[0:BL, 0:BL])
            ysb = spool.tile([4, BL], F32, tag="ysb")
            nc.vector.tensor_copy(out=ysb, in_=yT)

            if STAGE == 3 and i == 0:
                diag(ysb[0:1, :])
                return

            if i < 2:
                qps = psmall.tile([H, BL], F32, tag="ps")
                nc.tensor.matmul(qps, Gsb[:, i, :], ysb, start=True, stop=True)
                Qsb = spool.tile([H, BL], F32, tag="qsb")
                nc.vector.tensor_scalar(out=Qsb, in0=qps, scalar1=cvec[:, i + 1 : i + 2],
                                        scalar2=None, op0=ALU.add)

        # ================= output head =================
        hyps = psmall.tile([H, BL], F32, tag="ps")
        nc.tensor.matmul(hyps, Aec[:, 2, :], ysb, start=True, stop=True)
        hy_sb = spool.tile([H, BL], F32, tag="hy")
        nc.vector.tensor_copy(out=hy_sb, in_=hyps)
        o1 = psmall.tile([H, BL], F32, tag="ps")
        nc.tensor.matmul(o1, W1T, hy_sb, start=True, stop=True)
        r1 = spool.tile([H, BL], F32, tag="r1")
        nc.scalar.activation(out=r1, in_=o1, func=AF.Relu, bias=b1col[:, 0:1])
        o2 = psmall.tile([1, BL], F32, tag="ps")
        nc.tensor.matmul(o2, W2col, r1, start=True, stop=True)
        res = spool.tile([1, BL], F32, tag="res")
        nc.scalar.activation(out=res, in_=o2, func=AF.Identity, bias=b2sb[:, 0:1])
        nc.sync.dma_start(out=d["d_out"].ap().rearrange("b one -> one b"), in_=res)


def kernel(**inputs) -> np.ndarray:
    global _PROGRAM
    if _PROGRAM is None:
        _PROGRAM = _build_program()
    nc = _PROGRAM

    full = {k: np.asarray(v, dtype=np.float32) for k, v in inputs.items()}
    in_maps = []
    for c in range(NCORES):
        sl = slice(c * BL, (c + 1) * BL)
        m = dict(full)
        m["state_init"] = full["state_init"][sl]
        m["static"] = full["static"][sl]
        m["static_ch_l"] = full["static_ch_l"][sl]
        in_maps.append(m)

    res = run_bass_kernel_spmd(nc, in_maps, list(range(NCORES)))
    global _LAST_RESULTS
    _LAST_RESULTS = res
    outs = [res.results[c]["out"] for c in range(NCORES)]
    return np.concatenate(outs, axis=0).astype(np.float32)


_LAST_RESULTS = None
